# revision 13
# baseline (speedup 1.0000x reference)
"""Trainium2 Bass kernel for nn_AttentionBlock (B=2, C=256, D=H=W=16).

Pipeline: GroupNorm(8) -> 1x1x1 conv QKV -> single-head attention over
N=4096 spatial tokens -> 1x1x1 conv proj -> residual.

Sharding: 8 cores = 2 batches x 4 query-chunks of 1024 tokens.  Each core
computes group-norm stats + V' for its full batch (redundantly across the
4 cores sharing a batch) and attention only for its 1024 queries.

v2: fp8 DoubleRow rewrite.  All heavy matmuls run as fp8e4m3 DoubleRow
(2 rows/cycle, 256-deep contraction per instruction, 4x the f32r rate):

  - scores = x8_k^T (Mh8 xq~8 + u): the host sends Mh = Wk^T Wq; on device
    Mh8 gets the q-side GroupNorm scale folded per partition, T1 = Mh8 @ x~q
    gets the k-side scale + u (= Wk^T(Wq shift + qb)) folded in its
    PSUM->SBUF copy.  This removes the K and Q matmuls and copies entirely;
    scores use the raw fp8 x (host-quantized) as the stationary.
  - probabilities: exp(s/16 - 1.25) in fp8e4m3 (the global -1.25 shift
    cancels in softmax, keeps exp below fp8 inf).  ACT uses the native Exp
    (scale/bias folded into the activation); DVE computes the same thing
    with one tensor_scalar: uint8 bits = round(s*log2e*8/16 + B), bit-cast
    to fp8e4m3 = piecewise-linear exp2 (and uint8 saturation clamps the
    negative tail to +0).  The two engines split the 32 exp tiles.
  - out = P^T V'' accumulated 256 keys/instruction; V'' = (proj@Wv) xn with
    a ones-column for the softmax denominator; fbias is added to the
    residual instead (sum p/Z = 1 folds it out of the attention matmul).
  - normalize via DVE reciprocal+mul to bf16, transpose [q,ch]->[ch,q] via
    the DMA XBAR (SBUF->SBUF, frees PE + lets the residual add run on
    GPSIMD which cannot touch PSUM), + (x_q + fbias) on GPSIMD, bf16 DMA
    out (host widens to f32).

Stats/scale/shift chain as v1 (DVE bn_stats + ACT accum + GPSIMD scalar
chain incl. bit-trick rsqrt), but on bf16 x (half the DMA).
"""

import os
import sys

import numpy as np
import ml_dtypes

if "/opt/trn_rl_repo" not in sys.path:
    sys.path.insert(0, "/opt/trn_rl_repo")

import concourse.bass as bass
import concourse.mybir as mybir
import concourse.tile as tile
from concourse.bass_utils import run_bass_kernel_spmd

F32 = mybir.dt.float32
F32R = mybir.dt.float32r
F8 = mybir.dt.float8e4
F8E5 = mybir.dt.float8e5
BF16 = mybir.dt.bfloat16
U8 = mybir.dt.uint8
U16 = mybir.dt.uint16
I32 = mybir.dt.int32
AF = mybir.ActivationFunctionType
DR = mybir.MatmulPerfMode.DoubleRow
ALU = mybir.AluOpType

F8NP = ml_dtypes.float8_e4m3
BFNP = ml_dtypes.bfloat16

B = 2
C = 256
N = 4096          # D*H*W tokens
NQ = 1024         # queries per core
G = 8             # groupnorm groups
GS = C // G       # 32 channels per group (== DVE transpose block size)
EPS = 1e-5
NCORES = 8

SHIFT = 1.5                        # global score shift, cancels in softmax
# ACT tiles: native exp -> fp8e4m3.  DVE tiles: one tensor_scalar producing
# uint8 bits = round(s*A + B), bitcast fp8E5M2 (32-octave range keeps the
# bits in [0,123] for |score| <= 140, so the uint8 wrap/NaN region is
# unreachable).
EXPA = float(np.log2(np.e) * 4.0 / 16.0)
EXPB = float(60.0 - SHIFT * 4.0 * np.log2(np.e))

# exp engine pattern per qt (16 entries): 'A' = ACT native exp,
# 'D' = DVE bit-trick.
# per-PAIR exp engine ('A' ACT exp fp8e4m3 / 'D' DVE trick fp8e5m2); both
# j of a pair must share a dtype because the out-matmul reads the pair as
# one lhsT.  DVE is measured ~1.76x cheaper per tile but carries more side
# work; 9D/7A per 16 pairs.
PAIR_PAT = ["D", "A", "D", "D", "A", "D", "A", "D",
            "D", "A", "D", "A", "D", "A", "D", "A"]
EXP_PAT = [PAIR_PAT[j // 2] for j in range(32)]
# V' pair-copy engines (16 pairs)
VCOPY_PAT = ["A", "D"] * 8

LAST_RESULT = None
SPLIT = True


_WS_CTR = [0]


def split_waits(nc, cap=1):
    """walrus (this build) allows a single sync wait per instruction; move
    excess sync_info.on_wait entries onto same-engine NoOps inserted before
    the instruction."""
    for fn in nc.m.functions:
        for blk in fn.blocks:
            out = []
            changed = False
            for ins in blk.instructions:
                si = ins.sync_info
                waits = list(si.on_wait) if si is not None else []
                if len(waits) > cap:
                    for i in range(0, len(waits) - cap, cap):
                        nop = mybir.InstNoOp(
                            name=f"I-waitsplit-{_WS_CTR[0]}",
                            engine=ins.engine,
                            ins=[], outs=[],
                        )
                        nop.sync_info = mybir.SyncInfo(
                            on_wait=waits[i:i + cap], on_update=[]
                        )
                        _WS_CTR[0] += 1
                        out.append(nop)
                    ins.sync_info = mybir.SyncInfo(
                        on_wait=waits[len(waits) - cap:],
                        on_update=list(si.on_update),
                    )
                    changed = True
                out.append(ins)
            if changed:
                blk.instructions = out


def build_bass(reps=1):
    nc = bass.Bass(trn_type="TRN2")

    # ---- DRAM I/O ----
    xh_d = nc.dram_tensor("xh", [2, 128, N], U16, kind="ExternalInput")
    x8_d = nc.dram_tensor("x8", [128, 2, N], U8, kind="ExternalInput")
    # packed f32 consts per partition:
    # wqT0 256 | wqT1 256 | wpvT0 256 | wpvT1 256 | wkF0 256 | wkF1 256
    # | scb0 4 | scb1 4  => 1544 cols
    cst_d = nc.dram_tensor("cst", [128, 1544], F32, kind="ExternalInput")
    mh_d = nc.dram_tensor("mh", [128, 2, 256], U16, kind="ExternalInput")
    idb_d = nc.dram_tensor("idb", [128, 128], U16, kind="ExternalInput")
    out_d = nc.dram_tensor("out", [2, 128, NQ], U16, kind="ExternalOutput")

    with tile.TileContext(nc) as tc:
        with (
            tc.tile_pool(name="dbuf", bufs=2) as dbuf,
            tc.tile_pool(name="small", bufs=4) as small,
            tc.tile_pool(name="pe2p", bufs=5) as pe2p,
            tc.tile_pool(name="aop", bufs=2) as aop,
            tc.tile_pool(name="aotp", bufs=4) as aotp,
            tc.tile_pool(name="psJ", bufs=3, space="PSUM") as psJ,
            tc.tile_pool(name="psO", bufs=4, space="PSUM") as psO,
            tc.tile_pool(name="psV", bufs=1, space="PSUM") as psV,
        ):
            for _rep in range(reps):
                qoff = None  # per-core query offset handled host-side: the
                # host packs this core's query slice as xh columns qoff..;
                # device code uses a fixed window passed via layout (see
                # _host_prep): x8 column qoff+q equals xq column q.

                # preload the exp ACT table while DMAs run
                wtab = small.tile([128, 1], F32, tag="wtab")
                nc.vector.memset(wtab, 0.0)
                nc.scalar.activation(out=wtab, in_=wtab, func=AF.Exp)

                # ---- const loads ----
                idb = dbuf.tile([128, 128], U16, tag="idb", name="idb")
                nc.sync.dma_start(out=idb, in_=idb_d[:])
                idb_bf = idb.bitcast(BF16)
                cst = dbuf.tile([128, 1544], F32, tag="cst", name="cst")
                nc.sync.dma_start(out=cst, in_=cst_d[:])
                wqT = [cst[:, 0:256], cst[:, 256:512]]
                wpvT = [cst[:, 512:768], cst[:, 768:1024]]
                wkF = [cst[:, 1024:1280], cst[:, 1280:1536]]
                scb = [cst[:, 1536:1540], cst[:, 1540:1544]]
                mh = dbuf.tile([128, 2, 256], U16, tag="mh", name="mh")
                nc.sync.dma_start(out=mh, in_=mh_d[:])

                # ---- x loads (bf16) interleaved with stats ----
                xh = [dbuf.tile([128, N], U16, tag=f"xh{t}", name=f"xh{t}")
                      for t in range(2)]
                sts = [small.tile([128, 8, 6], F32, tag=f"bnst{t}",
                                  name=f"bnst{t}") for t in range(2)]
                acc = [small.tile([128, 2], F32, tag=f"acc{t}", name=f"acc{t}")
                       for t in range(2)]
                warm_i = [0]

                def warm():
                    wj = psV.tile([128, 2, 256], F32, tag="psV",
                                  name=f"warm{warm_i[0]}")
                    nc.tensor.matmul(
                        wj[:, 0, 0:128], lhsT=idb_bf, rhs=idb_bf,
                        start=True, stop=True, skip_group_check=True,
                    )
                    warm_i[0] += 1

                warm()
                for ch in range(4):
                    for t in range(2):
                        nc.sync.dma_start(
                            out=xh[t][:, ch * 1024:(ch + 1) * 1024],
                            in_=xh_d[t, :, ch * 1024:(ch + 1) * 1024],
                        )
                        chunk_bf = xh[t].bitcast(BF16)[
                            :, ch * 1024:(ch + 1) * 1024]
                        if ch < 3:
                            xrc = xh[t].bitcast(BF16).rearrange(
                                "p (s c) -> p s c", c=512
                            )
                            for i in (2 * ch, 2 * ch + 1):
                                nc.vector.bn_stats(
                                    out=sts[t][:, i, :], in_=xrc[:, i, :]
                                )
                        else:
                            j1 = small.tile([128, 1024], BF16, tag="actjunk")
                            nc.scalar.activation(
                                out=j1, in_=chunk_bf, func=AF.Copy,
                                accum_out=acc[t][:, 0:1],
                            )
                            j2 = small.tile([128, 1024], BF16, tag="actjunk")
                            nc.scalar.activation(
                                out=j2, in_=chunk_bf, func=AF.Square,
                                accum_out=acc[t][:, 1:2],
                            )
                    warm()
                x8t = dbuf.tile([128, 2, N], U8, tag="x8t", name="x8t")
                nc.sync.dma_start(out=x8t, in_=x8_d[:])
                x8f = x8t.bitcast(F8)

                scale = []
                shift = []
                for t in range(2):
                    mv = small.tile([128, 2], F32, tag="mv")
                    nc.vector.bn_aggr(out=mv, in_=sts[t][:, 0:6, :])
                    # combine: bn covers 3072 elems, ACT sums cover 1024
                    meanp = small.tile([128, 1], F32, tag="meanp")
                    nc.gpsimd.tensor_scalar(
                        meanp, acc[t][:, 0:1], 1.0 / N, None, ALU.mult
                    )
                    tmpm = small.tile([128, 1], F32, tag="tmpm")
                    nc.gpsimd.tensor_scalar(
                        tmpm, mv[:, 0:1], 3072.0 / N, None, ALU.mult
                    )
                    nc.gpsimd.tensor_add(meanp, meanp, tmpm)
                    e2 = small.tile([128, 1], F32, tag="e2")
                    nc.gpsimd.tensor_mul(e2, mv[:, 0:1], mv[:, 0:1])
                    nc.gpsimd.tensor_add(e2, e2, mv[:, 1:2])
                    nc.gpsimd.tensor_scalar(
                        e2, e2, 3072.0 / N, None, ALU.mult
                    )
                    tmpe = small.tile([128, 1], F32, tag="tmpe")
                    nc.gpsimd.tensor_scalar(
                        tmpe, acc[t][:, 1:2], 1.0 / N, None, ALU.mult
                    )
                    nc.gpsimd.tensor_add(e2, e2, tmpe)
                    # group sums via 32-block transpose
                    pp2 = small.tile([128, 2, GS], F32, tag="pp2")
                    nc.gpsimd.tensor_copy(
                        pp2[:, 0, :], meanp.to_broadcast([128, GS])
                    )
                    nc.gpsimd.tensor_copy(
                        pp2[:, 1, :], e2.to_broadcast([128, GS]))
                    tr2 = small.tile([128, 2, GS], F32, tag="tr2")
                    nc.vector.transpose(tr2.rearrange("p a b -> p (a b)"),
                                        pp2.rearrange("p a b -> p (a b)"))
                    red = small.tile([128, 2], F32, tag="red")
                    nc.vector.reduce_sum(red, tr2, axis=mybir.AxisListType.X)
                    inv32 = 1.0 / GS
                    mean_c = small.tile([128, 1], F32, tag="meanc")
                    nc.gpsimd.tensor_scalar_mul(mean_c, red[:, 0:1], inv32)
                    ve = small.tile([128, 1], F32, tag="ve")
                    nc.gpsimd.tensor_mul(ve, mean_c, mean_c)
                    nc.gpsimd.tensor_scalar(
                        ve, ve, -1.0, None, ALU.mult
                    )
                    nc.gpsimd.tensor_scalar(
                        red[:, 1:2], red[:, 1:2], inv32, EPS,
                        ALU.mult, ALU.add,
                    )
                    nc.gpsimd.tensor_add(ve, ve, red[:, 1:2])
                    # rstd = rsqrt(ve): bit-trick seed + 2 Newton steps
                    magic = small.tile([128, 1], I32, tag="magic")
                    nc.vector.memset(magic, 0x5F3759DF)
                    sh1 = small.tile([128, 1], I32, tag="sh1")
                    nc.vector.memset(sh1, 1)
                    yb = small.tile([128, 1], I32, tag="yb")
                    nc.vector.tensor_tensor(
                        yb, ve.bitcast(I32), sh1,
                        op=ALU.logical_shift_right,
                    )
                    nc.vector.tensor_tensor(
                        yb, magic, yb, op=ALU.subtract
                    )
                    y = yb.bitcast(F32)
                    t2 = small.tile([128, 1], F32, tag="t2")
                    for _ in range(2):
                        nc.gpsimd.tensor_mul(t2, y, y)
                        nc.gpsimd.tensor_mul(t2, t2, ve)
                        nc.gpsimd.tensor_scalar(
                            t2, t2, -0.5, 1.5,
                            ALU.mult, ALU.add,
                        )
                        nc.gpsimd.tensor_mul(y, y, t2)
                    # scale = rstd * norm_w ; shift = norm_b - mean*scale
                    sc = dbuf.tile([128, 1], F32, tag=f"scale{t}",
                                   name=f"scale{t}")
                    nc.gpsimd.tensor_mul(sc, y, scb[t][:, 2:3])
                    sh = dbuf.tile([128, 1], F32, tag=f"shift{t}",
                                   name=f"shift{t}")
                    nc.gpsimd.tensor_mul(sh, mean_c, sc)
                    nc.gpsimd.tensor_sub(sh, scb[t][:, 3:4], sh)
                    scale.append(sc)
                    shift.append(sh)

                # ---- weight folds (fp8) ----
                # mh8s[:, t, :] = fp8(mh_bf16[:, t, :] * scale[t])  (q-side)
                mh8s = dbuf.tile([128, 2, 256], U8, tag="mh8s", name="mh8s")
                for t in range(2):
                    nc.gpsimd.tensor_scalar_mul(
                        mh8s.bitcast(F8)[:, t, :], mh.bitcast(BF16)[:, t, :],
                        scale[t],
                    )
                wpv8 = dbuf.tile([128, 2, 256], U8, tag="wpv8", name="wpv8")
                for t in range(2):
                    nc.gpsimd.tensor_scalar_mul(
                        wpv8.bitcast(F8)[:, t, :], wpvT[t], scale[t],
                    )

                # ---- small bias matmuls ----
                # qv[m] = sum_t wqT[t][:,m]^T shift[t] + qb0[m]
                qv = []
                for m in range(2):
                    ps = psV.tile([128, 2, 256], F32, tag="psV",
                                  name=f"qvp{m}")
                    for t in range(2):
                        nc.tensor.matmul(
                            ps[:, 0, 0:1],
                            lhsT=wqT[t][:, m * 128:(m + 1) * 128],
                            rhs=shift[t],
                            start=(t == 0), stop=(t == 1),
                        )
                    qm = small.tile([128, 1], F32, tag=f"qv{m}", name=f"qv{m}")
                    nc.vector.tensor_add(qm, ps[:, 0, 0:1], scb[m][:, 0:1])
                    qv.append(qm)
                # u[h] = sum_m wkF[m][:,h]^T qv[m]
                u = []
                for h in range(2):
                    ps = psV.tile([128, 2, 256], F32, tag="psV",
                                  name=f"up{h}")
                    for m in range(2):
                        nc.tensor.matmul(
                            ps[:, 0, 0:1],
                            lhsT=wkF[m][:, h * 128:(h + 1) * 128],
                            rhs=qv[m],
                            start=(m == 0), stop=(m == 1),
                        )
                    uh = small.tile([128, 1], F32, tag=f"u{h}", name=f"u{h}")
                    nc.vector.tensor_copy(uh, ps[:, 0, 0:1])
                    u.append(uh)
                # fb[m] = sum_t wpvT[t][:,m]^T shift[t] + cb[m]
                fb = []
                for m in range(2):
                    ps = psV.tile([128, 2, 256], F32, tag="psV",
                                  name=f"fbp{m}")
                    for t in range(2):
                        nc.tensor.matmul(
                            ps[:, 0, 0:1],
                            lhsT=wpvT[t][:, m * 128:(m + 1) * 128],
                            rhs=shift[t],
                            start=(t == 0), stop=(t == 1),
                        )
                    fm = small.tile([128, 1], F32, tag=f"fb{m}", name=f"fb{m}")
                    nc.vector.tensor_add(fm, ps[:, 0, 0:1], scb[m][:, 1:2])
                    fb.append(fm)

                # xq'[m] = x[m-half, qslice] + fb[m]  (bf16, residual+fbias)
                xqr = []
                for m in range(2):
                    xr = dbuf.tile([128, NQ], U16, tag=f"xqr{m}",
                                   name=f"xqr{m}")
                    nc.gpsimd.tensor_scalar_add(
                        xr.bitcast(BF16), xh[m].bitcast(BF16)[:, 0:NQ],
                        fb[m],
                    )
                    xqr.append(xr)

                # exp bias tile for ACT
                nb = small.tile([128, 1], F32, tag="nb")
                nc.vector.memset(nb, -SHIFT)

                # ---- T1 = scale * (Mh8s @ x~q + u) ----
                # host layout guarantees this core's queries are x8 columns
                # 0..NQ (see _host_prep roll) -- use x8 cols 0..1024.
                T18 = dbuf.tile([128, 2, NQ], U8, tag="T18", name="T18")
                for h in range(2):
                    for c2 in range(NQ // 512):
                        psf = psJ.tile([128, 512], F32, tag="psJ",
                                       name=f"t1p{h}{c2}")
                        nc.tensor.matmul(
                            psf,
                            lhsT=mh8s.bitcast(F8)[:, :, h * 128:(h + 1) * 128],
                            rhs=x8f[:, :, c2 * 512:(c2 + 1) * 512],
                            start=True, stop=True, perf_mode=DR,
                        )
                        nc.vector.tensor_scalar(
                            T18.bitcast(F8)[:, h, c2 * 512:(c2 + 1) * 512],
                            psf, u[h], scale[h], ALU.add, ALU.mult,
                        )

                # ---- V'' tiles ----
                VT8 = dbuf.tile([128, 32, 257], U8, tag="VT8", name="VT8")
                nc.gpsimd.memset(VT8[:, :, 256], 0x38)  # fp8(1.0)

                vnext = [0]

                def emit_vpair():
                    j = vnext[0]
                    if j >= 32:
                        return
                    vnext[0] = j + 2
                    ps = psV.tile([128, 2, 256], F32, tag="psV",
                                  name=f"vp{j}")
                    for par in range(2):
                        nc.tensor.matmul(
                            ps[:, par, :],
                            lhsT=x8f[:, :, (j + par) * 128:(j + par + 1) * 128],
                            rhs=wpv8.bitcast(F8),
                            start=True, stop=True, perf_mode=DR,
                        )
                    eng = VCOPY_PAT[j // 2]
                    dst = VT8.bitcast(F8)[:, j:j + 2, 0:256]
                    if eng == "A":
                        nc.scalar.activation(
                            out=dst, in_=ps, func=AF.Copy,
                        )
                    else:
                        nc.vector.tensor_copy(dst, ps)

                for _ in range(3):
                    emit_vpair()

                # ---- attention ----
                fin = [dbuf.tile([128, NQ], U16, tag=f"fin{m}", name=f"fin{m}")
                       for m in range(2)]

                for qt in range(NQ // 512):
                    po = [psO.tile([128, 512], F32, tag="psO",
                                   name=f"po{qt}_{qs}") for qs in range(4)]

                    def emit_scores(j, qt=qt):
                        ps = psJ.tile([128, 512], F32, tag="psJ",
                                      name=f"ss{qt}_{j}")
                        nc.tensor.matmul(
                            ps,
                            lhsT=x8f[:, :, j * 128:(j + 1) * 128],
                            rhs=T18.bitcast(F8)[
                                :, :, qt * 512:(qt + 1) * 512],
                            start=True, stop=True, perf_mode=DR,
                        )
                        return ps

                    def emit_po(jj, pe2, po=po):
                        pdt = F8 if EXP_PAT[2 * jj] == "A" else F8E5
                        for qs in range(4):
                            nc.tensor.matmul(
                                po[qs][:, 0:257],
                                lhsT=pe2.bitcast(pdt)[
                                    :, :, qs * 128:(qs + 1) * 128],
                                rhs=VT8.bitcast(F8)[:, 2 * jj:2 * jj + 2, :],
                                start=(jj == 0), stop=(jj == 15),
                                perf_mode=DR,
                            )

                    pend = []
                    ss = emit_scores(0)
                    pe2 = None
                    for j in range(32):
                        if j % 2 == 0:
                            pe2 = pe2p.tile([128, 2, 512], U8, tag="pe2",
                                            name=f"pe2_{qt}_{j // 2}")
                        dst8 = pe2.bitcast(F8)[:, j % 2, :]
                        dstu = pe2[:, j % 2, :]
                        if EXP_PAT[j] == "A":
                            nc.scalar.activation(
                                out=dst8, in_=ss,
                                func=AF.Exp, scale=1.0 / 16.0, bias=nb,
                            )
                        else:
                            nc.vector.tensor_scalar(
                                dstu, ss, EXPA, EXPB,
                                ALU.mult, ALU.add,
                            )
                        if j < 31:
                            ss = emit_scores(j + 1)
                        if qt == 0 and j % 2 == 0:
                            emit_vpair()
                        if j % 2 == 1:
                            pend.append((j // 2, pe2))
                            if len(pend) > 2:
                                emit_po(*pend.pop(0))
                    for e in pend:
                        emit_po(*e)

                    # normalize -> bf16 [q, ch], DMA-transpose, +residual
                    for qs in range(4):
                        zr = small.tile([128, 1], F32, tag="zr")
                        nc.vector.reciprocal(zr, po[qs][:, 256:257])
                        ao = aop.tile([128, 256], BF16, tag="ao",
                                      name=f"ao{qt}_{qs}")
                        nc.vector.tensor_scalar_mul(ao, po[qs][:, 0:256], zr)
                        col = qt * 512 + qs * 128
                        for m in range(2):
                            aot = aotp.tile([128, 128], BF16, tag="aot",
                                            name=f"aot{qt}_{qs}_{m}")
                            nc.scalar.dma_start(
                                out=aot, in_=ao[:, m * 128:(m + 1) * 128],
                                transpose=True,
                            )
                            nc.gpsimd.tensor_tensor(
                                fin[m].bitcast(BF16)[:, col:col + 128],
                                aot, xqr[m].bitcast(BF16)[:, col:col + 128],
                                op=ALU.add,
                            )
                    for m in range(2):
                        cs = qt * 512
                        nc.scalar.dma_start(
                            out=out_d[m, :, cs:cs + 512],
                            in_=fin[m][:, cs:cs + 512],
                        )

    if SPLIT:
        split_waits(nc)
    return nc


_CACHED = {}
_RUNNER = {}


def _get_nc(reps=1):
    if reps not in _CACHED:
        _CACHED[reps] = build_bass(reps)
    return _CACHED[reps]


def _get_runner(reps=1):
    """Cached jitted shard_map runner over 8 cores (mirrors
    bass2jax.run_bass_via_pjrt, minus donation, so the compiled executable
    and device-resident inputs can be reused across calls)."""
    if reps in _RUNNER:
        return _RUNNER[reps]
    import jax
    from jax.experimental.shard_map import shard_map
    from jax.sharding import Mesh, PartitionSpec
    from concourse import bass2jax, mybir as mb
    from concourse.bass2jax import _bass_exec_p, install_neuronx_cc_hook

    nc = _get_nc(reps)
    install_neuronx_cc_hook()
    assert nc.dbg_addr is None
    partition_name = nc.partition_id_tensor.name if nc.partition_id_tensor else None

    in_names = []
    out_names = []
    out_avals = []
    zero_outs = []
    for alloc in nc.m.functions[0].allocations:
        if not isinstance(alloc, mb.MemoryLocationSet):
            continue
        name = alloc.memorylocations[0].name
        if alloc.kind == "ExternalInput":
            if name != partition_name:
                in_names.append(name)
        elif alloc.kind == "ExternalOutput":
            out_names.append(name)
            shape = tuple(alloc.tensor_shape)
            dtype = mb.dt.np(alloc.dtype)
            out_avals.append(jax.core.ShapedArray(shape, dtype))
            zero_outs.append(np.zeros(shape, dtype))
    n_params = len(in_names)
    all_in_names = in_names + out_names
    if partition_name is not None:
        all_in_names = all_in_names + [partition_name]

    def _body(*args):
        operands = list(args)
        if partition_name is not None:
            operands.append(bass2jax.partition_id_tensor())
        outs = _bass_exec_p.bind(
            *operands,
            out_avals=tuple(out_avals),
            in_names=tuple(all_in_names),
            out_names=tuple(out_names),
            lowering_input_output_aliases=(),
            sim_require_finite=True,
            sim_require_nnan=True,
            nc=nc,
        )
        return tuple(outs)

    devices = jax.devices()[:NCORES]
    mesh = Mesh(np.asarray(devices), ("core",))
    n_outs = len(out_names)
    sharded = jax.jit(
        shard_map(
            _body,
            mesh=mesh,
            in_specs=(PartitionSpec("core"),) * (n_params + n_outs),
            out_specs=(PartitionSpec("core"),) * n_outs,
            check_rep=False,
        ),
        keep_unused=True,
    )
    _RUNNER[reps] = (sharded, in_names, out_names, out_avals, zero_outs, mesh)
    return _RUNNER[reps]


def _concat_inputs(in_maps, in_names, zero_outs):
    concat_in = [
        np.concatenate([np.asarray(in_maps[c][name]) for c in range(NCORES)], axis=0)
        for name in in_names
    ]
    concat_zeros = [
        np.zeros((NCORES * z.shape[0], *z.shape[1:]), z.dtype) for z in zero_outs
    ]
    return concat_in, concat_zeros


def _run(in_maps):
    sharded, in_names, out_names, out_avals, zero_outs, mesh = _get_runner()
    concat_in, concat_zeros = _concat_inputs(in_maps, in_names, zero_outs)
    out_arrs = sharded(*concat_in, *concat_zeros)
    return [
        {
            name: np.asarray(out_arrs[i]).reshape(NCORES, *out_avals[i].shape)[c]
            for i, name in enumerate(out_names)
        }
        for c in range(NCORES)
    ]


def _host_prep(x, norm_w, norm_b, qkv_w, qkv_b, proj_w, proj_b):
    wq = qkv_w[0:C]
    wk = qkv_w[C:2 * C]
    wv = qkv_w[2 * C:3 * C]
    wpv = (proj_w @ wv).astype(np.float32)
    mh = (wk.T @ wq).astype(np.float32)           # [ck, cq]
    qb0 = qkv_b[0:C]
    cb = (proj_w @ qkv_b[2 * C:3 * C] + proj_b).astype(np.float32)

    wqT = np.ascontiguousarray(wq.T).reshape(2, 128, 256)
    wpvT = np.ascontiguousarray(wpv.T).reshape(2, 128, 256)
    wkF = wk.reshape(2, 128, 256)
    scb = np.stack(
        [qb0.reshape(2, 128), cb.reshape(2, 128),
         norm_w.reshape(2, 128), norm_b.reshape(2, 128)], axis=2,
    ).astype(np.float32)                          # [2, 128, 4]
    cst = np.ascontiguousarray(np.concatenate(
        [wqT[0], wqT[1], wpvT[0], wpvT[1], wkF[0], wkF[1],
         scb[0], scb[1]], axis=1,
    )).astype(np.float32)

    # mh bf16 in lhsT layout: mh_l[p, t, ck] = mh[ck, t*128+p]
    mh_l = np.ascontiguousarray(
        mh.T.reshape(2, 128, 256).transpose(1, 0, 2).astype(BFNP)
    ).view(np.uint16)

    idb = np.ascontiguousarray(np.eye(128, dtype=BFNP)).view(np.uint16)

    xf = x.reshape(B, 2, 128, N)
    in_maps = []
    for core in range(NCORES):
        b, qi = divmod(core, NCORES // B)
        # roll the token axis so this core's 1024 queries are columns 0..1024
        xr = np.roll(xf[b], -qi * NQ, axis=2)
        xh = np.ascontiguousarray(xr.astype(BFNP)).view(np.uint16)
        x8 = np.ascontiguousarray(
            xr.astype(F8NP).transpose(1, 0, 2)).view(np.uint8)
        in_maps.append(
            {"xh": xh, "x8": x8, "cst": cst, "mh": mh_l, "idb": idb}
        )
    return in_maps


def kernel(x, norm_w, norm_b, qkv_w, qkv_b, proj_w, proj_b):
    x = np.ascontiguousarray(np.asarray(x, dtype=np.float32))
    norm_w = np.asarray(norm_w, dtype=np.float32)
    norm_b = np.asarray(norm_b, dtype=np.float32)
    qkv_w = np.asarray(qkv_w, dtype=np.float32)
    qkv_b = np.asarray(qkv_b, dtype=np.float32)
    proj_w = np.asarray(proj_w, dtype=np.float32)
    proj_b = np.asarray(proj_b, dtype=np.float32)

    Bs, Cs = x.shape[0], x.shape[1]
    assert (Bs, Cs) == (B, C) and x.shape[2] * x.shape[3] * x.shape[4] == N

    in_maps = _host_prep(x, norm_w, norm_b, qkv_w, qkv_b, proj_w, proj_b)
    results = _run(in_maps)

    y = np.empty((B, C, N), dtype=np.float32)
    for core in range(NCORES):
        b, qi = divmod(core, NCORES // B)
        ob = results[core]["out"].view(BFNP).astype(np.float32)
        y[b, :, qi * NQ:(qi + 1) * NQ] = ob.reshape(C, NQ)
    return y.reshape(x.shape)


def bench(in_maps, iters=50, warmup=3, reps=1):
    """Amortized per-execution device time: device-resident inputs, back-to-
    back async executes, block at the end."""
    import time
    import jax
    from jax.sharding import NamedSharding, PartitionSpec

    sharded, in_names, out_names, out_avals, zero_outs, mesh = _get_runner(reps)
    concat_in, concat_zeros = _concat_inputs(in_maps, in_names, zero_outs)
    sh = NamedSharding(mesh, PartitionSpec("core"))
    dev_in = [jax.device_put(a, sh) for a in concat_in]
    dev_zero = [jax.device_put(a, sh) for a in concat_zeros]
    for _ in range(warmup):
        out = sharded(*dev_in, *dev_zero)
    jax.block_until_ready(out)
    t0 = time.perf_counter()
    for _ in range(iters):
        out = sharded(*dev_in, *dev_zero)
    jax.block_until_ready(out)
    t1 = time.perf_counter()
    return (t1 - t0) / iters


# revision 15
# speedup vs baseline: 1.0947x; 1.0947x over previous
"""Trainium2 Bass kernel for nn_AttentionBlock (B=2, C=256, D=H=W=16).

Pipeline: GroupNorm(8) -> 1x1x1 conv QKV -> single-head attention over
N=4096 spatial tokens -> 1x1x1 conv proj -> residual.

Sharding: 8 cores = 2 batches x 4 query-chunks of 1024 tokens.  Each core
computes group-norm stats + V' for its full batch (redundantly across the
4 cores sharing a batch) and attention only for its 1024 queries.

v2: fp8 DoubleRow rewrite.  All heavy matmuls run as fp8e4m3 DoubleRow
(2 rows/cycle, 256-deep contraction per instruction, 4x the f32r rate):

  - scores = x8_k^T (Mh8 xq~8 + u): the host sends Mh = Wk^T Wq; on device
    Mh8 gets the q-side GroupNorm scale folded per partition, T1 = Mh8 @ x~q
    gets the k-side scale + u (= Wk^T(Wq shift + qb)) folded in its
    PSUM->SBUF copy.  This removes the K and Q matmuls and copies entirely;
    scores use the raw fp8 x (host-quantized) as the stationary.
  - probabilities: exp(s/16 - 1.25) in fp8e4m3 (the global -1.25 shift
    cancels in softmax, keeps exp below fp8 inf).  ACT uses the native Exp
    (scale/bias folded into the activation); DVE computes the same thing
    with one tensor_scalar: uint8 bits = round(s*log2e*8/16 + B), bit-cast
    to fp8e4m3 = piecewise-linear exp2 (and uint8 saturation clamps the
    negative tail to +0).  The two engines split the 32 exp tiles.
  - out = P^T V'' accumulated 256 keys/instruction; V'' = (proj@Wv) xn with
    a ones-column for the softmax denominator; fbias is added to the
    residual instead (sum p/Z = 1 folds it out of the attention matmul).
  - normalize via DVE reciprocal+mul to bf16, transpose [q,ch]->[ch,q] via
    the DMA XBAR (SBUF->SBUF, frees PE + lets the residual add run on
    GPSIMD which cannot touch PSUM), + (x_q + fbias) on GPSIMD, bf16 DMA
    out (host widens to f32).

Stats/scale/shift chain as v1 (DVE bn_stats + ACT accum + GPSIMD scalar
chain incl. bit-trick rsqrt), but on bf16 x (half the DMA).
"""

import os
import sys

import numpy as np
import ml_dtypes

if "/opt/trn_rl_repo" not in sys.path:
    sys.path.insert(0, "/opt/trn_rl_repo")

import concourse.bass as bass
import concourse.mybir as mybir
import concourse.tile as tile
from concourse.bass_utils import run_bass_kernel_spmd

F32 = mybir.dt.float32
F32R = mybir.dt.float32r
F8 = mybir.dt.float8e4
F8E5 = mybir.dt.float8e5
BF16 = mybir.dt.bfloat16
U8 = mybir.dt.uint8
U16 = mybir.dt.uint16
I32 = mybir.dt.int32
AF = mybir.ActivationFunctionType
DR = mybir.MatmulPerfMode.DoubleRow
ALU = mybir.AluOpType

F8NP = ml_dtypes.float8_e4m3
BFNP = ml_dtypes.bfloat16

B = 2
C = 256
N = 4096          # D*H*W tokens
NQ = 1024         # queries per core
G = 8             # groupnorm groups
GS = C // G       # 32 channels per group (== DVE transpose block size)
EPS = 1e-5
NCORES = 8

SHIFT = 1.5                        # global score shift, cancels in softmax
# ACT tiles: native exp -> fp8e4m3.  DVE tiles: one tensor_scalar producing
# uint8 bits = round(s*A + B), bitcast fp8E5M2 (32-octave range keeps the
# bits in [0,123] for |score| <= 140, so the uint8 wrap/NaN region is
# unreachable).
EXPA = float(np.log2(np.e) * 4.0 / 16.0)
EXPB = float(60.0 - SHIFT * 4.0 * np.log2(np.e))

# exp engine pattern per qt (16 entries): 'A' = ACT native exp,
# 'D' = DVE bit-trick.
# per-PAIR exp engine ('A' ACT exp fp8e4m3 / 'D' DVE trick fp8e5m2); both
# j of a pair must share a dtype because the out-matmul reads the pair as
# one lhsT.  DVE is measured ~1.76x cheaper per tile but carries more side
# work; 9D/7A per 16 pairs.
PAIR_PAT = ["D", "A", "D", "D", "A", "D", "A", "D",
            "D", "A", "D", "A", "D", "A", "D", "A"]
EXP_PAT = [PAIR_PAT[j // 2] for j in range(32)]
# V' pair-copy engines (16 pairs)
VCOPY_PAT = ["A", "D"] * 8

LAST_RESULT = None
SPLIT = True


_WS_CTR = [0]


def split_waits(nc, cap=1):
    """walrus (this build) allows a single sync wait per instruction; move
    excess sync_info.on_wait entries onto same-engine NoOps inserted before
    the instruction."""
    for fn in nc.m.functions:
        for blk in fn.blocks:
            out = []
            changed = False
            for ins in blk.instructions:
                si = ins.sync_info
                waits = list(si.on_wait) if si is not None else []
                if len(waits) > cap:
                    for i in range(0, len(waits) - cap, cap):
                        nop = mybir.InstNoOp(
                            name=f"I-waitsplit-{_WS_CTR[0]}",
                            engine=ins.engine,
                            ins=[], outs=[],
                        )
                        nop.sync_info = mybir.SyncInfo(
                            on_wait=waits[i:i + cap], on_update=[]
                        )
                        _WS_CTR[0] += 1
                        out.append(nop)
                    ins.sync_info = mybir.SyncInfo(
                        on_wait=waits[len(waits) - cap:],
                        on_update=list(si.on_update),
                    )
                    changed = True
                out.append(ins)
            if changed:
                blk.instructions = out


def build_bass(reps=1):
    nc = bass.Bass(trn_type="TRN2")

    # ---- DRAM I/O ----
    xh_d = nc.dram_tensor("xh", [2, 128, N], U16, kind="ExternalInput")
    x8_d = nc.dram_tensor("x8", [128, 2, N], U8, kind="ExternalInput")
    # packed f32 consts per partition:
    # wqT0 256 | wqT1 256 | wpvT0 256 | wpvT1 256 | wkF0 256 | wkF1 256
    # | scb0 4 | scb1 4  => 1544 cols
    cst_d = nc.dram_tensor("cst", [128, 1544], F32, kind="ExternalInput")
    mh_d = nc.dram_tensor("mh", [128, 2, 256], U16, kind="ExternalInput")
    idb_d = nc.dram_tensor("idb", [128, 128], U16, kind="ExternalInput")
    out_d = nc.dram_tensor("out", [2, 128, NQ], U16, kind="ExternalOutput")

    with tile.TileContext(nc) as tc:
        with (
            tc.tile_pool(name="dbuf", bufs=2) as dbuf,
            tc.tile_pool(name="small", bufs=4) as small,
            tc.tile_pool(name="pe2p", bufs=5) as pe2p,
            tc.tile_pool(name="aop", bufs=2) as aop,
            tc.tile_pool(name="aotp", bufs=4) as aotp,
            tc.tile_pool(name="psJ", bufs=3, space="PSUM") as psJ,
            tc.tile_pool(name="psO", bufs=4, space="PSUM") as psO,
            tc.tile_pool(name="psV", bufs=1, space="PSUM") as psV,
        ):
            for _rep in range(reps):
                qoff = None  # per-core query offset handled host-side: the
                # host packs this core's query slice as xh columns qoff..;
                # device code uses a fixed window passed via layout (see
                # _host_prep): x8 column qoff+q equals xq column q.

                # preload the exp ACT table while DMAs run
                wtab = small.tile([128, 1], F32, tag="wtab")
                nc.vector.memset(wtab, 0.0)
                nc.scalar.activation(out=wtab, in_=wtab, func=AF.Exp)

                # ---- const loads ----
                idb = dbuf.tile([128, 128], U16, tag="idb", name="idb")
                nc.scalar.dma_start(out=idb, in_=idb_d[:])
                idb_bf = idb.bitcast(BF16)
                cst = dbuf.tile([128, 1544], F32, tag="cst", name="cst")
                nc.scalar.dma_start(out=cst, in_=cst_d[:])
                wqT = [cst[:, 0:256], cst[:, 256:512]]
                wpvT = [cst[:, 512:768], cst[:, 768:1024]]
                wkF = [cst[:, 1024:1280], cst[:, 1280:1536]]
                scb = [cst[:, 1536:1540], cst[:, 1540:1544]]
                mh = dbuf.tile([128, 2, 256], U16, tag="mh", name="mh")
                nc.scalar.dma_start(out=mh, in_=mh_d[:])

                # ---- x loads (bf16) interleaved with stats ----
                xh = [dbuf.tile([128, N], U16, tag=f"xh{t}", name=f"xh{t}")
                      for t in range(2)]
                sts = [small.tile([128, 8, 6], F32, tag=f"bnst{t}",
                                  name=f"bnst{t}") for t in range(2)]
                acc = [small.tile([128, 2], F32, tag=f"acc{t}", name=f"acc{t}")
                       for t in range(2)]
                warm_i = [0]

                def warm():
                    wj = psV.tile([128, 2, 256], F32, tag="psV",
                                  name=f"warm{warm_i[0]}")
                    nc.tensor.matmul(
                        wj[:, 0, 0:128], lhsT=idb_bf, rhs=idb_bf,
                        start=True, stop=True, skip_group_check=True,
                    )
                    warm_i[0] += 1

                warm()
                for ch in range(4):
                    for t in range(2):
                        nc.scalar.dma_start(
                            out=xh[t][:, ch * 1024:(ch + 1) * 1024],
                            in_=xh_d[t, :, ch * 1024:(ch + 1) * 1024],
                        )
                        chunk_bf = xh[t].bitcast(BF16)[
                            :, ch * 1024:(ch + 1) * 1024]
                        if ch < 3:
                            xrc = xh[t].bitcast(BF16).rearrange(
                                "p (s c) -> p s c", c=512
                            )
                            for i in (2 * ch, 2 * ch + 1):
                                nc.vector.bn_stats(
                                    out=sts[t][:, i, :], in_=xrc[:, i, :]
                                )
                        else:
                            j1 = small.tile([128, 1024], BF16, tag="actjunk")
                            nc.scalar.activation(
                                out=j1, in_=chunk_bf, func=AF.Copy,
                                accum_out=acc[t][:, 0:1],
                            )
                            j2 = small.tile([128, 1024], BF16, tag="actjunk")
                            nc.scalar.activation(
                                out=j2, in_=chunk_bf, func=AF.Square,
                                accum_out=acc[t][:, 1:2],
                            )
                    warm()
                x8t = dbuf.tile([128, 2, N], U8, tag="x8t", name="x8t")
                nc.scalar.dma_start(out=x8t, in_=x8_d[:])
                x8f = x8t.bitcast(F8)

                scale = []
                shift = []
                for t in range(2):
                    mv = small.tile([128, 2], F32, tag="mv")
                    nc.vector.bn_aggr(out=mv, in_=sts[t][:, 0:6, :])
                    # combine: bn covers 3072 elems, ACT sums cover 1024
                    meanp = small.tile([128, 1], F32, tag="meanp")
                    nc.gpsimd.tensor_scalar(
                        meanp, acc[t][:, 0:1], 1.0 / N, None, ALU.mult
                    )
                    tmpm = small.tile([128, 1], F32, tag="tmpm")
                    nc.gpsimd.tensor_scalar(
                        tmpm, mv[:, 0:1], 3072.0 / N, None, ALU.mult
                    )
                    nc.gpsimd.tensor_add(meanp, meanp, tmpm)
                    e2 = small.tile([128, 1], F32, tag="e2")
                    nc.gpsimd.tensor_mul(e2, mv[:, 0:1], mv[:, 0:1])
                    nc.gpsimd.tensor_add(e2, e2, mv[:, 1:2])
                    nc.gpsimd.tensor_scalar(
                        e2, e2, 3072.0 / N, None, ALU.mult
                    )
                    tmpe = small.tile([128, 1], F32, tag="tmpe")
                    nc.gpsimd.tensor_scalar(
                        tmpe, acc[t][:, 1:2], 1.0 / N, None, ALU.mult
                    )
                    nc.gpsimd.tensor_add(e2, e2, tmpe)
                    # group sums via 32-block transpose
                    pp2 = small.tile([128, 2, GS], F32, tag="pp2")
                    nc.gpsimd.tensor_copy(
                        pp2[:, 0, :], meanp.to_broadcast([128, GS])
                    )
                    nc.gpsimd.tensor_copy(
                        pp2[:, 1, :], e2.to_broadcast([128, GS]))
                    tr2 = small.tile([128, 2, GS], F32, tag="tr2")
                    nc.vector.transpose(tr2.rearrange("p a b -> p (a b)"),
                                        pp2.rearrange("p a b -> p (a b)"))
                    red = small.tile([128, 2], F32, tag="red")
                    nc.vector.reduce_sum(red, tr2, axis=mybir.AxisListType.X)
                    inv32 = 1.0 / GS
                    mean_c = small.tile([128, 1], F32, tag="meanc")
                    nc.gpsimd.tensor_scalar_mul(mean_c, red[:, 0:1], inv32)
                    ve = small.tile([128, 1], F32, tag="ve")
                    nc.gpsimd.tensor_mul(ve, mean_c, mean_c)
                    nc.gpsimd.tensor_scalar(
                        ve, ve, -1.0, None, ALU.mult
                    )
                    nc.gpsimd.tensor_scalar(
                        red[:, 1:2], red[:, 1:2], inv32, EPS,
                        ALU.mult, ALU.add,
                    )
                    nc.gpsimd.tensor_add(ve, ve, red[:, 1:2])
                    # rstd = rsqrt(ve): bit-trick seed + 2 Newton steps
                    magic = small.tile([128, 1], I32, tag="magic")
                    nc.vector.memset(magic, 0x5F3759DF)
                    sh1 = small.tile([128, 1], I32, tag="sh1")
                    nc.vector.memset(sh1, 1)
                    yb = small.tile([128, 1], I32, tag="yb")
                    nc.vector.tensor_tensor(
                        yb, ve.bitcast(I32), sh1,
                        op=ALU.logical_shift_right,
                    )
                    nc.vector.tensor_tensor(
                        yb, magic, yb, op=ALU.subtract
                    )
                    y = yb.bitcast(F32)
                    t2 = small.tile([128, 1], F32, tag="t2")
                    for _ in range(2):
                        nc.gpsimd.tensor_mul(t2, y, y)
                        nc.gpsimd.tensor_mul(t2, t2, ve)
                        nc.gpsimd.tensor_scalar(
                            t2, t2, -0.5, 1.5,
                            ALU.mult, ALU.add,
                        )
                        nc.gpsimd.tensor_mul(y, y, t2)
                    # scale = rstd * norm_w ; shift = norm_b - mean*scale
                    sc = dbuf.tile([128, 1], F32, tag=f"scale{t}",
                                   name=f"scale{t}")
                    nc.gpsimd.tensor_mul(sc, y, scb[t][:, 2:3])
                    sh = dbuf.tile([128, 1], F32, tag=f"shift{t}",
                                   name=f"shift{t}")
                    nc.gpsimd.tensor_mul(sh, mean_c, sc)
                    nc.gpsimd.tensor_sub(sh, scb[t][:, 3:4], sh)
                    scale.append(sc)
                    shift.append(sh)

                # ---- weight folds (fp8) ----
                # mh8s[:, t, :] = fp8(mh_bf16[:, t, :] * scale[t])  (q-side)
                mh8s = dbuf.tile([128, 2, 256], U8, tag="mh8s", name="mh8s")
                for t in range(2):
                    nc.gpsimd.tensor_scalar_mul(
                        mh8s.bitcast(F8)[:, t, :], mh.bitcast(BF16)[:, t, :],
                        scale[t],
                    )
                wpv8 = dbuf.tile([128, 2, 256], U8, tag="wpv8", name="wpv8")
                for t in range(2):
                    nc.gpsimd.tensor_scalar_mul(
                        wpv8.bitcast(F8)[:, t, :], wpvT[t], scale[t],
                    )

                # ---- small bias matmuls ----
                # qv[m] = sum_t wqT[t][:,m]^T shift[t] + qb0[m]
                qv = []
                for m in range(2):
                    ps = psV.tile([128, 2, 256], F32, tag="psV",
                                  name=f"qvp{m}")
                    for t in range(2):
                        nc.tensor.matmul(
                            ps[:, 0, 0:1],
                            lhsT=wqT[t][:, m * 128:(m + 1) * 128],
                            rhs=shift[t],
                            start=(t == 0), stop=(t == 1),
                        )
                    qm = small.tile([128, 1], F32, tag=f"qv{m}", name=f"qv{m}")
                    nc.vector.tensor_add(qm, ps[:, 0, 0:1], scb[m][:, 0:1])
                    qv.append(qm)
                # u[h] = sum_m wkF[m][:,h]^T qv[m]
                u = []
                for h in range(2):
                    ps = psV.tile([128, 2, 256], F32, tag="psV",
                                  name=f"up{h}")
                    for m in range(2):
                        nc.tensor.matmul(
                            ps[:, 0, 0:1],
                            lhsT=wkF[m][:, h * 128:(h + 1) * 128],
                            rhs=qv[m],
                            start=(m == 0), stop=(m == 1),
                        )
                    uh = small.tile([128, 1], F32, tag=f"u{h}", name=f"u{h}")
                    nc.vector.tensor_copy(uh, ps[:, 0, 0:1])
                    u.append(uh)
                # fb[m] = sum_t wpvT[t][:,m]^T shift[t] + cb[m]
                fb = []
                for m in range(2):
                    ps = psV.tile([128, 2, 256], F32, tag="psV",
                                  name=f"fbp{m}")
                    for t in range(2):
                        nc.tensor.matmul(
                            ps[:, 0, 0:1],
                            lhsT=wpvT[t][:, m * 128:(m + 1) * 128],
                            rhs=shift[t],
                            start=(t == 0), stop=(t == 1),
                        )
                    fm = small.tile([128, 1], F32, tag=f"fb{m}", name=f"fb{m}")
                    nc.vector.tensor_add(fm, ps[:, 0, 0:1], scb[m][:, 1:2])
                    fb.append(fm)

                # xq'[m] = x[m-half, qslice] + fb[m]  (bf16, residual+fbias)
                xqr = []
                for m in range(2):
                    xr = dbuf.tile([128, NQ], U16, tag=f"xqr{m}",
                                   name=f"xqr{m}")
                    nc.gpsimd.tensor_scalar_add(
                        xr.bitcast(BF16), xh[m].bitcast(BF16)[:, 0:NQ],
                        fb[m],
                    )
                    xqr.append(xr)

                # exp bias tile for ACT
                nb = small.tile([128, 1], F32, tag="nb")
                nc.vector.memset(nb, -SHIFT)

                # ---- T1 = scale * (Mh8s @ x~q + u) ----
                # host layout guarantees this core's queries are x8 columns
                # 0..NQ (see _host_prep roll) -- use x8 cols 0..1024.
                T18 = dbuf.tile([128, 2, NQ], U8, tag="T18", name="T18")
                for h in range(2):
                    for c2 in range(NQ // 512):
                        psf = psJ.tile([128, 512], F32, tag="psJ",
                                       name=f"t1p{h}{c2}")
                        nc.tensor.matmul(
                            psf,
                            lhsT=mh8s.bitcast(F8)[:, :, h * 128:(h + 1) * 128],
                            rhs=x8f[:, :, c2 * 512:(c2 + 1) * 512],
                            start=True, stop=True, perf_mode=DR,
                        )
                        nc.vector.tensor_scalar(
                            T18.bitcast(F8)[:, h, c2 * 512:(c2 + 1) * 512],
                            psf, u[h], scale[h], ALU.add, ALU.mult,
                        )

                # ---- V'' tiles ----
                VT8 = dbuf.tile([128, 32, 257], U8, tag="VT8", name="VT8")
                nc.gpsimd.memset(VT8[:, :, 256], 0x38)  # fp8(1.0)

                for p in range(16):
                    j = 2 * p
                    ps = psJ.tile([128, 512], F32, tag="psJ", name=f"vp{j}")
                    psv2 = ps.rearrange("p (a b) -> p a b", a=2)
                    for par in range(2):
                        nc.tensor.matmul(
                            psv2[:, par, :],
                            lhsT=x8f[:, :, (j + par) * 128:(j + par + 1) * 128],
                            rhs=wpv8.bitcast(F8),
                            start=(par == 0), stop=(par == 1), perf_mode=DR,
                        )
                    dst = VT8.bitcast(F8)[:, j:j + 2, 0:256]
                    if VCOPY_PAT[p] == "A":
                        nc.scalar.activation(
                            out=dst, in_=psv2, func=AF.Copy,
                        )
                    else:
                        nc.vector.tensor_copy(dst, psv2)

                # ---- attention ----
                fin = [dbuf.tile([128, NQ], U16, tag=f"fin{m}", name=f"fin{m}")
                       for m in range(2)]

                deferred = []

                for qt in range(NQ // 512):
                    po = [psO.tile([128, 512], F32, tag="psO",
                                   name=f"po{qt}_{qs}") for qs in range(4)]

                    def emit_scores(j, qt=qt):
                        ps = psJ.tile([128, 512], F32, tag="psJ",
                                      name=f"ss{qt}_{j}")
                        nc.tensor.matmul(
                            ps,
                            lhsT=x8f[:, :, j * 128:(j + 1) * 128],
                            rhs=T18.bitcast(F8)[
                                :, :, qt * 512:(qt + 1) * 512],
                            start=True, stop=True, perf_mode=DR,
                        )
                        return ps

                    def emit_po(jj, pe2, po=po):
                        pdt = F8 if EXP_PAT[2 * jj] == "A" else F8E5
                        for qs in range(4):
                            nc.tensor.matmul(
                                po[qs][:, 0:257],
                                lhsT=pe2.bitcast(pdt)[
                                    :, :, qs * 128:(qs + 1) * 128],
                                rhs=VT8.bitcast(F8)[:, 2 * jj:2 * jj + 2, :],
                                start=(jj == 0), stop=(jj == 15),
                                perf_mode=DR,
                            )

                    pend = []
                    ss = emit_scores(0)
                    pe2 = None
                    for j in range(32):
                        if j % 2 == 0:
                            pe2 = pe2p.tile([128, 2, 512], U8, tag="pe2",
                                            name=f"pe2_{qt}_{j // 2}")
                        dst8 = pe2.bitcast(F8)[:, j % 2, :]
                        dstu = pe2[:, j % 2, :]
                        if EXP_PAT[j] == "A":
                            nc.scalar.activation(
                                out=dst8, in_=ss,
                                func=AF.Exp, scale=1.0 / 16.0, bias=nb,
                            )
                        else:
                            nc.vector.tensor_scalar(
                                dstu, ss, EXPA, EXPB,
                                ALU.mult, ALU.add,
                            )
                        if j < 31:
                            ss = emit_scores(j + 1)
                        if j % 2 == 1 and deferred:
                            deferred.pop(0)()
                        if j % 2 == 1:
                            pend.append((j // 2, pe2))
                            if len(pend) > 2:
                                emit_po(*pend.pop(0))
                    for e in pend:
                        emit_po(*e)

                    # normalize -> bf16 [q, ch], DMA-transpose, +residual
                    def make_final(qs, qt=qt, po=po):
                        def f():
                            zr = small.tile([128, 1], F32, tag="zr")
                            nc.vector.reciprocal(zr, po[qs][:, 256:257])
                            ao = aop.tile([128, 256], BF16, tag="ao",
                                          name=f"ao{qt}_{qs}")
                            nc.vector.tensor_scalar_mul(
                                ao, po[qs][:, 0:256], zr)
                            col = qt * 512 + qs * 128
                            for m in range(2):
                                aot = aotp.tile([128, 128], BF16, tag="aot",
                                                name=f"aot{qt}_{qs}_{m}")
                                nc.sync.dma_start(
                                    out=aot,
                                    in_=ao[:, m * 128:(m + 1) * 128],
                                    transpose=True,
                                )
                                nc.gpsimd.tensor_tensor(
                                    fin[m].bitcast(BF16)[:, col:col + 128],
                                    aot,
                                    xqr[m].bitcast(BF16)[:, col:col + 128],
                                    op=ALU.add,
                                )
                            if qs == 3:
                                for m in range(2):
                                    cs = qt * 512
                                    nc.sync.dma_start(
                                        out=out_d[m, :, cs:cs + 512],
                                        in_=fin[m][:, cs:cs + 512],
                                    )
                        return f

                    deferred.extend(make_final(qs) for qs in range(4))
                for f in deferred:
                    f()

    if SPLIT:
        split_waits(nc)
    return nc


_CACHED = {}
_RUNNER = {}


def _get_nc(reps=1):
    if reps not in _CACHED:
        _CACHED[reps] = build_bass(reps)
    return _CACHED[reps]


def _get_runner(reps=1):
    """Cached jitted shard_map runner over 8 cores (mirrors
    bass2jax.run_bass_via_pjrt, minus donation, so the compiled executable
    and device-resident inputs can be reused across calls)."""
    if reps in _RUNNER:
        return _RUNNER[reps]
    import jax
    from jax.experimental.shard_map import shard_map
    from jax.sharding import Mesh, PartitionSpec
    from concourse import bass2jax, mybir as mb
    from concourse.bass2jax import _bass_exec_p, install_neuronx_cc_hook

    nc = _get_nc(reps)
    install_neuronx_cc_hook()
    assert nc.dbg_addr is None
    partition_name = nc.partition_id_tensor.name if nc.partition_id_tensor else None

    in_names = []
    out_names = []
    out_avals = []
    zero_outs = []
    for alloc in nc.m.functions[0].allocations:
        if not isinstance(alloc, mb.MemoryLocationSet):
            continue
        name = alloc.memorylocations[0].name
        if alloc.kind == "ExternalInput":
            if name != partition_name:
                in_names.append(name)
        elif alloc.kind == "ExternalOutput":
            out_names.append(name)
            shape = tuple(alloc.tensor_shape)
            dtype = mb.dt.np(alloc.dtype)
            out_avals.append(jax.core.ShapedArray(shape, dtype))
            zero_outs.append(np.zeros(shape, dtype))
    n_params = len(in_names)
    all_in_names = in_names + out_names
    if partition_name is not None:
        all_in_names = all_in_names + [partition_name]

    def _body(*args):
        operands = list(args)
        if partition_name is not None:
            operands.append(bass2jax.partition_id_tensor())
        outs = _bass_exec_p.bind(
            *operands,
            out_avals=tuple(out_avals),
            in_names=tuple(all_in_names),
            out_names=tuple(out_names),
            lowering_input_output_aliases=(),
            sim_require_finite=True,
            sim_require_nnan=True,
            nc=nc,
        )
        return tuple(outs)

    devices = jax.devices()[:NCORES]
    mesh = Mesh(np.asarray(devices), ("core",))
    n_outs = len(out_names)
    sharded = jax.jit(
        shard_map(
            _body,
            mesh=mesh,
            in_specs=(PartitionSpec("core"),) * (n_params + n_outs),
            out_specs=(PartitionSpec("core"),) * n_outs,
            check_rep=False,
        ),
        keep_unused=True,
    )
    _RUNNER[reps] = (sharded, in_names, out_names, out_avals, zero_outs, mesh)
    return _RUNNER[reps]


def _concat_inputs(in_maps, in_names, zero_outs):
    concat_in = [
        np.concatenate([np.asarray(in_maps[c][name]) for c in range(NCORES)], axis=0)
        for name in in_names
    ]
    concat_zeros = [
        np.zeros((NCORES * z.shape[0], *z.shape[1:]), z.dtype) for z in zero_outs
    ]
    return concat_in, concat_zeros


def _run(in_maps):
    sharded, in_names, out_names, out_avals, zero_outs, mesh = _get_runner()
    concat_in, concat_zeros = _concat_inputs(in_maps, in_names, zero_outs)
    out_arrs = sharded(*concat_in, *concat_zeros)
    return [
        {
            name: np.asarray(out_arrs[i]).reshape(NCORES, *out_avals[i].shape)[c]
            for i, name in enumerate(out_names)
        }
        for c in range(NCORES)
    ]


def _host_prep(x, norm_w, norm_b, qkv_w, qkv_b, proj_w, proj_b):
    wq = qkv_w[0:C]
    wk = qkv_w[C:2 * C]
    wv = qkv_w[2 * C:3 * C]
    wpv = (proj_w @ wv).astype(np.float32)
    mh = (wk.T @ wq).astype(np.float32)           # [ck, cq]
    qb0 = qkv_b[0:C]
    cb = (proj_w @ qkv_b[2 * C:3 * C] + proj_b).astype(np.float32)

    wqT = np.ascontiguousarray(wq.T).reshape(2, 128, 256)
    wpvT = np.ascontiguousarray(wpv.T).reshape(2, 128, 256)
    wkF = wk.reshape(2, 128, 256)
    scb = np.stack(
        [qb0.reshape(2, 128), cb.reshape(2, 128),
         norm_w.reshape(2, 128), norm_b.reshape(2, 128)], axis=2,
    ).astype(np.float32)                          # [2, 128, 4]
    cst = np.ascontiguousarray(np.concatenate(
        [wqT[0], wqT[1], wpvT[0], wpvT[1], wkF[0], wkF[1],
         scb[0], scb[1]], axis=1,
    )).astype(np.float32)

    # mh bf16 in lhsT layout: mh_l[p, t, ck] = mh[ck, t*128+p]
    mh_l = np.ascontiguousarray(
        mh.T.reshape(2, 128, 256).transpose(1, 0, 2).astype(BFNP)
    ).view(np.uint16)

    idb = np.ascontiguousarray(np.eye(128, dtype=BFNP)).view(np.uint16)

    xf = x.reshape(B, 2, 128, N)
    in_maps = []
    for core in range(NCORES):
        b, qi = divmod(core, NCORES // B)
        # roll the token axis so this core's 1024 queries are columns 0..1024
        xr = np.roll(xf[b], -qi * NQ, axis=2)
        xh = np.ascontiguousarray(xr.astype(BFNP)).view(np.uint16)
        x8 = np.ascontiguousarray(
            xr.astype(F8NP).transpose(1, 0, 2)).view(np.uint8)
        in_maps.append(
            {"xh": xh, "x8": x8, "cst": cst, "mh": mh_l, "idb": idb}
        )
    return in_maps


def kernel(x, norm_w, norm_b, qkv_w, qkv_b, proj_w, proj_b):
    x = np.ascontiguousarray(np.asarray(x, dtype=np.float32))
    norm_w = np.asarray(norm_w, dtype=np.float32)
    norm_b = np.asarray(norm_b, dtype=np.float32)
    qkv_w = np.asarray(qkv_w, dtype=np.float32)
    qkv_b = np.asarray(qkv_b, dtype=np.float32)
    proj_w = np.asarray(proj_w, dtype=np.float32)
    proj_b = np.asarray(proj_b, dtype=np.float32)

    Bs, Cs = x.shape[0], x.shape[1]
    assert (Bs, Cs) == (B, C) and x.shape[2] * x.shape[3] * x.shape[4] == N

    in_maps = _host_prep(x, norm_w, norm_b, qkv_w, qkv_b, proj_w, proj_b)
    results = _run(in_maps)

    y = np.empty((B, C, N), dtype=np.float32)
    for core in range(NCORES):
        b, qi = divmod(core, NCORES // B)
        ob = results[core]["out"].view(BFNP).astype(np.float32)
        y[b, :, qi * NQ:(qi + 1) * NQ] = ob.reshape(C, NQ)
    return y.reshape(x.shape)


def bench(in_maps, iters=50, warmup=3, reps=1):
    """Amortized per-execution device time: device-resident inputs, back-to-
    back async executes, block at the end."""
    import time
    import jax
    from jax.sharding import NamedSharding, PartitionSpec

    sharded, in_names, out_names, out_avals, zero_outs, mesh = _get_runner(reps)
    concat_in, concat_zeros = _concat_inputs(in_maps, in_names, zero_outs)
    sh = NamedSharding(mesh, PartitionSpec("core"))
    dev_in = [jax.device_put(a, sh) for a in concat_in]
    dev_zero = [jax.device_put(a, sh) for a in concat_zeros]
    for _ in range(warmup):
        out = sharded(*dev_in, *dev_zero)
    jax.block_until_ready(out)
    t0 = time.perf_counter()
    for _ in range(iters):
        out = sharded(*dev_in, *dev_zero)
    jax.block_until_ready(out)
    t1 = time.perf_counter()
    return (t1 - t0) / iters
